# revision 8
# baseline (speedup 1.0000x reference)
"""Causal multi-head attention on 8 Trainium2 cores (raw Bass).

Problem: x[4,2048,1024] @ W_qkv -> 16-head causal attention -> @ W_proj.
Sharding: core c handles batch b=c//2 and head-half c%2 (8 heads each).
Host pre-transposes x (feature-major xT, bf16) and pre-slices/scales
weights; each core computes its heads' contribution to out^T; host sums
the two half contributions per batch and adds b_proj.

v2 pipeline (bf16 attention operands, fp32 PSUM, head-PAIR waves):
  The 8 heads form 4 pairs g; head h=2g lives on partitions 0:64 and
  h=2g+1 on 64:128 of qk_sb.  Per (g, q-chunk) the S^T matmuls of both
  heads issue back-to-back with 64-row operands at base partition 0/64,
  so the PE runs them CONCURRENTLY on disjoint row-groups (2x on S).
  exp on ACT covers both heads' PSUM banks in one instruction; DVE
  applies the causal triangle on diagonal 128-blocks post-exp (bf16 4x
  mode).  PV accumulates y_aug^T per head (ones-column gives softmax
  sums), reciprocal + K=1 replication matmuls + DVE multiply normalize.

  Cross-phase software pipelining: A1 (q,k projection) is emitted
  per-pair; pair g+1's A1 matmuls are interleaved into pair g's
  attention waves as PE filler (PSUM banks 6/7, shared with the
  replication matmuls via WAR semaphores) so the PE never stalls while
  ACT catches up on exp.  The C phase (out-proj) units for q-chunks
  0..2 interleave into the last pair's attention the same way.

build_nc(t, reps) can replicate the whole pipeline `reps` times inside
one NEFF (serialized at rep boundaries) for wall-clock timing dilation.
"""

import contextlib
import math

import numpy as np
import ml_dtypes

import concourse.bass as bass
import concourse.mybir as mybir
from concourse.bass_utils import run_bass_kernel_spmd

F32 = mybir.dt.float32
F32R = mybir.dt.float32r
BF16 = mybir.dt.bfloat16
ADD = mybir.AluOpType.add
MULT = mybir.AluOpType.mult
EXP = mybir.ActivationFunctionType.Exp
COPY = mybir.ActivationFunctionType.Copy

D_MODEL = 1024
D_K = 64
B, T = 4, 2048
NH = 8          # heads per core
NP = 4          # head pairs per core
KC = 8          # D_MODEL / 128
TQ = 512        # q-chunk width
N_CORES = 8


def build_nc(t=T, reps=1):
    tt_n = t // 128
    tc_n = t // TQ
    nc = bass.Bass(target_bir_lowering=False)

    xT_d = nc.dram_tensor("xT", [128, KC, t], BF16, kind="ExternalInput")
    wqk_d = nc.dram_tensor("wqk", [128, KC, NP, 2, 128], BF16,
                           kind="ExternalInput")
    wv_d = nc.dram_tensor("wv", [128, KC, 512], BF16, kind="ExternalInput")
    wproj_d = nc.dram_tensor("wproj", [128, 4, 1024], F32R,
                             kind="ExternalInput")
    bqk_d = nc.dram_tensor("bqk", [128, 8], F32, kind="ExternalInput")
    bv_d = nc.dram_tensor("bv", [128, 512], F32, kind="ExternalInput")
    tri_d = nc.dram_tensor("tri", [128, 128], BF16, kind="ExternalInput")
    ones_d = nc.dram_tensor("onesv", [128, 64], F32R, kind="ExternalInput")
    out_d = nc.dram_tensor("outT", [128, 8, t], F32, kind="ExternalOutput")

    # ---- schedule state ----
    prog = {"sync": [], "tensor": [], "vector": [], "scalar": []}
    cnt = {"pe": 0, "act": 0, "dve": 0}
    for _c in range(8):
        cnt[f"dma{_c}"] = 0
    last_wait = {e: {} for e in prog}
    bank_war = {}          # psum bank -> (sem, value): last consumer finished
    FUSE = {"tensor", "vector", "scalar"}

    def op(engine, fn, waits=(), incs=()):
        w = []
        for s, v in waits:
            if v <= 0 or last_wait[engine].get(s, -1) >= v:
                continue
            last_wait[engine][s] = v
            w.append((s, v))
        prog[engine].append((fn, w, list(incs), engine in FUSE))
        for s, a in incs:
            cnt[s] += a

    NDMA = 8
    dma_rr = [0]

    def dma(dst, src, waits=()):
        ch = dma_rr[0] % NDMA
        dma_rr[0] += 1
        sem = f"dma{ch}"
        w = [(sem, cnt[sem])] + list(waits)   # chain within channel
        op("sync", lambda e, d=dst, s=src: e.dma_start(d, s),
           w, [(sem, 16)])
        return (sem, cnt[sem])

    stack = contextlib.ExitStack()
    sb = lambda name, shape, dt: stack.enter_context(
        nc.sbuf_tensor(name, shape, dt))

    xT_sb = sb("xT_sb", [128, KC, t], BF16)
    wqg = sb("wqg", [128, NP, KC, 2, 128], BF16)
    wv_buf = sb("wv_buf", [128, 4, 512], BF16)
    qk_sb = sb("qk", [128, 8, t], BF16)          # f: 0..3 q-pair, 4..7 k-pair
    v_sb = sb("vsb", [128, tt_n, 8, 65], BF16)
    pt_sb = sb("pt", [128, 8, 512], BF16)        # slot = 2*(kt%4) + head
    rsb = sb("rsb", [65, 2, 512], F32R)
    yun = sb("yun", [64, 2, 512], F32)
    ysb = sb("ysb", [128, 4, t], F32R)
    ysbt = sb("ysbt", [64, 2, t], F32R)
    osb = sb("osb", [128, 8, 512], F32)
    wproj_sb = sb("wproj_sb", [128, 4, 1024], F32R)
    bqk_sb = sb("bqk_sb", [128, 8], F32)
    bv_sb = sb("bv_sb", [128, 512], F32)
    tri_sb = sb("tri_sb", [128, 128], BF16)
    ones_sb = sb("ones_sb", [128, 64], F32R)
    psum = stack.enter_context(nc.psum_tensor("ps", [128, 8, 512], F32))

    with contextlib.ExitStack() as semstack:
        semstack.enter_context(nc.allow_low_precision(
            reason="bf16 matmul operands are intentional (checked rel err)"))
        sems = {}
        for _nm in ["pe", "act", "dve"] + [f"dma{_c}" for _c in range(8)]:
            sems[_nm] = semstack.enter_context(nc.semaphore(_nm + "_sem"))

        bqk_ret = dma(bqk_sb.ap(), bqk_d[:])
        bv_ret = dma(bv_sb.ap(), bv_d[:])
        tri_ret = dma(tri_sb.ap(), tri_d[:])
        ones_ret = dma(ones_sb.ap(), ones_d[:])

        # ones column of V_aug via DVE (x*0 + 1)
        op("vector",
           lambda e: e.tensor_scalar(
               v_sb.ap()[:, :, :, 64:65],
               bv_sb.ap()[:, 0:tt_n * 8].rearrange(
                   "p (a b c) -> p a b c", a=tt_n, b=8),
               0.0, 1.0, MULT, mybir.AluOpType.add),
           [bv_ret], [("dve", 1)])
        vones_ret = ("dve", cnt["dve"])

        ysbt_war = {}
        rep_gate = []

        for rep in range(reps):
            # ---- per-rep input DMAs ----
            xT_done = {}
            for kc in range(KC):
                xT_done[kc] = dma(xT_sb.ap()[:, kc], xT_d[:, kc],
                                  list(rep_gate))
            wqg_done = {}
            for g in range(NP):
                for kc in range(KC):
                    wqg_done[(g, kc)] = dma(
                        wqg.ap()[:, g, kc], wqk_d[:, kc, g], list(rep_gate))

            # ---------- A1 emission helper (one chain = 8 kc MMs + copy) ---
            qk_done = {}

            def a1_chain(g, tc, j, bank):
                """Emit full chain immediately (A-phase mode)."""
                for kc in range(KC):
                    a1_mm(g, tc, j, bank, kc)
                a1_copy(g, tc, j, bank)

            def a1_mm(g, tc, j, bank, kc):
                w = []
                if kc == 0:
                    w = [wqg_done[(g, kc)], xT_done[kc]]
                    if bank in bank_war:
                        w.append(bank_war.pop(bank))
                else:
                    w = [wqg_done[(g, kc)], xT_done[kc]]
                op("tensor",
                   lambda e, b=bank, g_=g, k=kc, j_=j, tc_=tc:
                       e.matmul(
                           psum.ap()[:, b],
                           wqg.ap()[:, g_, k, j_],
                           xT_sb.ap()[:, k, tc_ * TQ:(tc_ + 1) * TQ],
                           start=(k == 0), stop=(k == KC - 1)),
                   w, [("pe", 1)] if kc == KC - 1 else [])

            def a1_copy(g, tc, j, bank):
                f = g + 4 * j
                mm_done = cnt["pe"]
                op("vector",
                   lambda e, b=bank, f_=f, tc_=tc:
                       e.tensor_scalar(
                           qk_sb.ap()[:, f_, tc_ * TQ:(tc_ + 1) * TQ],
                           psum.ap()[:, b],
                           bqk_sb.ap()[:, f_:f_ + 1], None, ADD),
                   [("pe", mm_done), bqk_ret], [("dve", 1)])
                bank_war[bank] = ("dve", cnt["dve"])
                qk_done[g] = cnt["dve"]

            # ---- A1 for pair 0 (dedicated phase, banks 0..3) ----
            for tc in range(tc_n):
                for j in range(2):
                    a1_chain(0, tc, j, 2 * (tc % 2) + j)

            # ---- A2: V projection, all pairs (baseline structure, bf16) ---
            tt_groups = [list(range(i, min(i + 4, tt_n)))
                         for i in range(0, tt_n, 4)]
            wv_seq = 0
            a2_kc_done = {}
            for tg, tts in enumerate(tt_groups):
                for kc in range(KC):
                    slot = wv_seq % 4
                    war = list(rep_gate)
                    if wv_seq - 4 >= 0:
                        war.append(("pe", a2_kc_done[wv_seq - 4]))
                    nd = dma(wv_buf.ap()[:, slot], wv_d[:, kc], war)
                    for j, tt in enumerate(tts):
                        bank = (tg % 2) * 4 + j
                        w = [nd, xT_done[kc]] if j == 0 else [xT_done[kc]]
                        if kc == 0 and bank in bank_war:
                            w.append(bank_war.pop(bank))
                        op("tensor",
                           lambda e, b=bank, s=slot, k=kc, tt_=tt:
                               e.matmul(
                                   psum.ap()[:, b],
                                   xT_sb.ap()[:, k, tt_ * 128:(tt_ + 1) * 128],
                                   wv_buf.ap()[:, s],
                                   start=(k == 0), stop=(k == KC - 1)),
                           w, [("pe", 1)] if j == len(tts) - 1 else [])
                    a2_kc_done[wv_seq] = cnt["pe"]
                    wv_seq += 1
                grp_done = cnt["pe"]
                for j, tt in enumerate(tts):
                    bk = (tg % 2) * 4 + j
                    op("vector",
                       lambda e, b=bk, tt_=tt:
                           e.tensor_tensor(
                               v_sb.ap()[:, tt_, :, 0:64],
                               psum.ap()[:, b], bv_sb.ap()[:], ADD),
                       [("pe", grp_done), bv_ret], [("dve", 1)])
                    bank_war[bk] = ("dve", cnt["dve"])
            a2_copies = cnt["dve"]
            a2_pe_done = cnt["pe"]

            wproj_dma = dma(wproj_sb.ap(), wproj_d[:],
                            [("pe", a2_pe_done)] + list(rep_gate))

            # ---------- filler queue (A1 of pair g+1, or C units) ----------
            filler_items = []     # list of closures, each emits ONE PE op

            def load_a1_filler(g):
                # chains alternate banks 6/7; copy emitted with 8th MM
                for ci, (tc, j) in enumerate(
                        [(tc, j) for tc in range(tc_n) for j in range(2)]):
                    bank = 6 + (ci % 2)
                    for kc in range(KC):
                        def item(g=g, tc=tc, j=j, bank=bank, kc=kc):
                            a1_mm(g, tc, j, bank, kc)
                            if kc == KC - 1:
                                a1_copy(g, tc, j, bank)
                        filler_items.append(item)

            def emit_filler(n):
                for _ in range(n):
                    if not filler_items:
                        return
                    filler_items.pop(0)()

            # ---------- C-phase unit ----------
            c_copy = {}
            c_dma = {}
            c_seq = [0]
            norm_gate = {}        # tc -> (dve cnt, [shift rets]) needed by C

            def c_unit(ft, tc, bank):
                j = c_seq[0]
                dv, shifts = norm_gate[tc]
                w = [("dve", dv), wproj_dma] + shifts
                if bank in bank_war:
                    w.append(bank_war.pop(bank))
                if j >= 8:
                    w.append(("act", c_copy[j - 8]))
                for gg in range(4):
                    op("tensor",
                       lambda e, bk=bank, g_=gg, f=ft, tc_=tc: e.matmul(
                           psum.ap()[:, bk],
                           wproj_sb.ap()[:, g_, f * 128:(f + 1) * 128],
                           ysb.ap()[:, g_, tc_ * TQ:(tc_ + 1) * TQ],
                           start=(g_ == 0), stop=(g_ == 3)),
                       w if gg == 0 else [],
                       [("pe", 1)] if gg == 3 else [])
                mm_done = cnt["pe"]
                w = [("pe", mm_done)]
                if j >= 8:
                    w.append(c_dma[j - 8])
                op("scalar",
                   lambda e, bk=bank, ob=j % 8: e.activation(
                       osb.ap()[:, ob], psum.ap()[:, bk], COPY),
                   w, [("act", 1)])
                c_copy[j] = cnt["act"]
                bank_war[bank] = ("act", cnt["act"])
                c_dma[j] = dma(
                    out_d[:, ft, tc * TQ:(tc + 1) * TQ],
                    osb.ap()[:, j % 8],
                    [("act", c_copy[j])])
                c_seq[0] += 1

            def load_c_filler(tc):
                for fi, ft in enumerate(range(8)):
                    def item(ft=ft, tc=tc, fi=fi):
                        c_unit(ft, tc, 6 + (fi % 2))
                    filler_items.append(item)

            # ---------- B phase: head pairs ----------
            s_exp = {}            # kt -> act cnt of exp
            slot_war = {}         # pt slot -> pe cnt of last PV read
            tri_done = {}         # (0|1) -> dve cnt  (r01 / r23 masks)
            ybank_war = {}        # pv bank -> (sem, val)
            shifts_by_tc = {}     # tc -> [shift dma rets] across pairs
            first_pv = [True]

            for g in range(NP):
                while filler_items:
                    emit_filler(8)
                if g < NP - 1:
                    load_a1_filler(g + 1)
                for qc in range(tc_n):
                    nkt = 4 * qc + 4
                    s_exp.clear()
                    tri_done.clear()

                    def s_wave(kt, qc=qc, g=g):
                        bA = 2 * (kt % 2)
                        r = kt - 4 * qc
                        off = max(0, r * 128)
                        n = TQ - off
                        w = [("dve", qk_done[g])]
                        for b in (bA, bA + 1):
                            if b in bank_war:
                                w.append(bank_war.pop(b))
                        if kt - 2 in s_exp:
                            w.append(("act", s_exp[kt - 2]))
                        for hh in (0, 1):
                            qrow = hh * 64
                            op("tensor",
                               lambda e, b=bA + hh, qrow=qrow, kt=kt,
                                      off=off, n=n, g=g:
                                   e.matmul(
                                       psum.ap()[:, b, off:off + n],
                                       qk_sb.ap()[qrow:qrow + 64, 4 + g,
                                                  kt * 128:(kt + 1) * 128],
                                       qk_sb.ap()[qrow:qrow + 64, g,
                                                  qc * TQ + off:
                                                  qc * TQ + off + n],
                                       start=True, stop=True),
                               w if hh == 0 else [],
                               [("pe", 1)] if hh == 1 else [])
                        s_done = cnt["pe"]
                        # exp over both heads' banks -> pt slots
                        sl = 2 * (kt % 4)
                        w = [("pe", s_done)]
                        if sl in slot_war:
                            w.append(("pe", slot_war[sl]))
                        op("scalar",
                           lambda e, bA=bA, sl=sl, off=off, n=n:
                               e.activation(
                                   pt_sb.ap()[:, sl:sl + 2, off:off + n],
                                   psum.ap()[:, bA:bA + 2, off:off + n],
                                   EXP),
                           w, [("act", 1)])
                        s_exp[kt] = cnt["act"]
                        bank_war[bA] = ("act", cnt["act"])
                        bank_war[bA + 1] = ("act", cnt["act"])
                        # triangle masks after exps of diag rows r=0,1 / 2,3
                        if r == 1 or r == 3:
                            half = r // 2
                            for rr in (r - 1, r):
                                for hh in (0, 1):
                                    slr = 2 * ((4 * qc + rr) % 4) + hh
                                    op("vector",
                                       lambda e, s=slr, rr=rr:
                                           e.tensor_tensor(
                                               pt_sb.ap()[:, s, rr * 128:
                                                          rr * 128 + 128],
                                               pt_sb.ap()[:, s, rr * 128:
                                                          rr * 128 + 128],
                                               tri_sb.ap()[:], MULT),
                                       [("act", s_exp[4 * qc + rr]), tri_ret],
                                       [("dve", 1)])
                            tri_done[half] = cnt["dve"]

                    def pv_pair(kt, qc=qc, g=g):
                        r = kt - 4 * qc
                        off = max(0, r * 128)
                        n = TQ - off
                        w = [("act", s_exp[kt])]
                        if r >= 0:
                            w.append(("dve", tri_done[max(0, r) // 2]))
                        if first_pv[0]:
                            w += [vones_ret, ("dve", a2_copies)]
                            first_pv[0] = False
                        if kt == 0:
                            for b in (4, 5):
                                if b in ybank_war:
                                    w.append(ybank_war[b])
                        for hh in (0, 1):
                            sl = 2 * (kt % 4) + hh
                            op("tensor",
                               lambda e, yb=4 + hh, h=2 * g + hh, sl=sl,
                                      kt=kt, off=off, n=n, nkt=nkt:
                                   e.matmul(
                                       psum.ap()[0:65, yb, off:off + n],
                                       v_sb.ap()[:, kt, h, :],
                                       pt_sb.ap()[:, sl, off:off + n],
                                       start=(kt == 0), stop=(kt == nkt - 1)),
                               w if hh == 0 else [],
                               [("pe", 1)] if hh == 1 else [])
                        slot_war[2 * (kt % 4)] = cnt["pe"]
                        slot_war[2 * (kt % 4) + 1] = cnt["pe"]

                    for kt in range(nkt):
                        emit_filler(2)
                        s_wave(kt)
                        if kt >= 2:
                            pv_pair(kt - 2)
                    emit_filler(2)
                    pv_pair(nkt - 2)
                    emit_filler(2)
                    pv_pair(nkt - 1)
                    pv_all = cnt["pe"]

                    # softmax sums -> reciprocal (both heads, one DVE op)
                    op("vector",
                       lambda e: e.reciprocal(
                           rsb.ap()[64:65, :, :], psum.ap()[64:65, 4:6, :]),
                       [("pe", pv_all)], [("dve", 1)])
                    recip_done = cnt["dve"]
                    # replication matmuls (banks 6,7 shared with filler)
                    emit_filler(1)
                    rep_pe = {}
                    for hh in (0, 1):
                        w = [("dve", recip_done), ones_ret]
                        if 6 + hh in bank_war:
                            w.append(bank_war.pop(6 + hh))
                        op("tensor",
                           lambda e, rb=6 + hh, hh=hh: e.matmul(
                               psum.ap()[0:64, rb],
                               ones_sb.ap()[64:65, :],
                               rsb.ap()[64:65, hh],
                               start=True, stop=True),
                           w, [("pe", 1)])
                        rep_pe[hh] = cnt["pe"]
                    # yun copies + normalize + per-qc ysbt shift
                    shift_w = []
                    for hh in (0, 1):
                        op("vector",
                           lambda e, yb=4 + hh, hh=hh: e.tensor_copy(
                               yun.ap()[0:64, hh].bitcast(F32),
                               psum.ap()[0:64, yb]),
                           [("pe", pv_all)], [("dve", 1)])
                        ybank_war[4 + hh] = ("dve", cnt["dve"])
                        if hh == 0:
                            out_ap = ysb.ap()[0:64, g, qc * TQ:(qc + 1) * TQ]
                        else:
                            out_ap = ysbt.ap()[0:64, g % 2,
                                               qc * TQ:(qc + 1) * TQ]
                        w = [("pe", rep_pe[hh])]
                        if hh == 1 and (g % 2, qc) in ysbt_war:
                            w.append(ysbt_war[(g % 2, qc)])
                        op("vector",
                           lambda e, o=out_ap, rb=6 + hh, hh=hh:
                               e.tensor_tensor(
                                   o, yun.ap()[0:64, hh].bitcast(F32),
                                   psum.ap()[0:64, rb], MULT),
                           w, [("dve", 1)])
                        bank_war[6 + hh] = ("dve", cnt["dve"])
                        if hh == 1:
                            nd = dma(ysb.ap()[64:128, g,
                                              qc * TQ:(qc + 1) * TQ],
                                     ysbt.ap()[0:64, g % 2,
                                               qc * TQ:(qc + 1) * TQ],
                                     [("dve", cnt["dve"])])
                            ysbt_war[(g % 2, qc)] = nd
                            shifts_by_tc.setdefault(qc, []).append(nd)
                    if g == NP - 1:
                        norm_gate[qc] = (cnt["dve"],
                                         list(shifts_by_tc[qc]))
                        if qc < tc_n - 1:
                            load_c_filler(qc)
                if g == NP - 1:
                    # drain any unemitted C filler, then tail tc
                    while filler_items:
                        emit_filler(8)

            # ---- C tail: last q-chunk on banks 0..3 ----
            for ft in range(8):
                c_unit(ft, tc_n - 1, ft % 4)

            rep_gate = [("act", c_copy[c_seq[0] - 1]), c_dma[c_seq[0] - 1]]
            # seed psum WARs for next rep (S banks last read by exp; PV by
            # yun copies; 6/7 by norms/C units)
            for bk in range(8):
                bank_war.setdefault(bk, rep_gate[0])

        # ---- emit ----
        with nc.Block() as block:
            def emitter(name):
                def run(eng):
                    for fn, waits, incs, fuse in prog[name]:
                        pre = waits[1:] if (fuse and waits) else waits
                        for s, v in pre:
                            eng.wait_ge(sems[s], v)
                        ins = fn(eng)
                        if fuse and waits:
                            s, v = waits[0]
                            ins.wait_op(sems[s], v, "sem-ge")
                        for s, a in incs:
                            ins.then_inc(sems[s], a)
                return run
            block.sync(emitter("sync"))
            block.tensor(emitter("tensor"))
            block.vector(emitter("vector"))
            block.scalar(emitter("scalar"))

    stack.close()
    return nc


# ---------------------------------------------------------------------------

def host_prep(x, W_qkv, b_qkv, W_proj, b_proj, t=T):
    scale = 1.0 / math.sqrt(D_K)
    x = np.asarray(x, np.float32)
    W_qkv = np.asarray(W_qkv, np.float32)
    b_qkv = np.asarray(b_qkv, np.float32)
    W_proj = np.asarray(W_proj, np.float32)
    bf = ml_dtypes.bfloat16

    tri = (np.arange(128)[None, :] >= np.arange(128)[:, None]) \
        .astype(bf)
    onesv = np.ones((128, 64), np.float32)

    in_maps = []
    for c in range(N_CORES):
        b = c // 2
        f0 = (c % 2) * 512
        xT = np.ascontiguousarray(
            x[b, :t].T.reshape(KC, 128, t).transpose(1, 0, 2)).astype(bf)
        wq = W_qkv[:, f0:f0 + 512] * scale
        wk = W_qkv[:, D_MODEL + f0:D_MODEL + f0 + 512]
        # [1024, NP, 2, 128]: per pair g the q slice then k slice
        wqk = np.stack([wq.reshape(D_MODEL, 4, 128),
                        wk.reshape(D_MODEL, 4, 128)], axis=2)
        wqk = np.ascontiguousarray(
            wqk.reshape(KC, 128, NP, 2, 128).transpose(1, 0, 2, 3, 4)
        ).astype(bf)
        wv = W_qkv[:, 2 * D_MODEL + f0:2 * D_MODEL + f0 + 512]
        wv = np.ascontiguousarray(
            wv.reshape(KC, 128, 512).transpose(1, 0, 2)).astype(bf)
        bq = b_qkv[f0:f0 + 512] * scale
        bk = b_qkv[D_MODEL + f0:D_MODEL + f0 + 512]
        bqk = np.ascontiguousarray(
            np.concatenate([bq, bk]).reshape(8, 128).T)
        bv = b_qkv[2 * D_MODEL + f0:2 * D_MODEL + f0 + 512]
        bv_rep = np.broadcast_to(bv, (128, 512)).copy()
        wp = W_proj[f0:f0 + 512]
        wp = np.ascontiguousarray(
            wp.reshape(4, 128, 1024).transpose(1, 0, 2))
        in_maps.append({
            "xT": xT, "wqk": wqk, "wv": wv, "wproj": wp,
            "bqk": bqk, "bv": bv_rep, "tri": tri, "onesv": onesv,
        })
    return in_maps


def host_gather(results, b_proj, t=T):
    b_proj = np.asarray(b_proj, np.float32)
    out = np.empty((B, t, D_MODEL), np.float32)
    for b in range(B):
        acc = None
        for half in range(2):
            r = results[2 * b + half]["outT"]
            oT = r.transpose(1, 0, 2).reshape(D_MODEL, t)
            acc = oT if acc is None else acc + oT
        out[b] = acc.T + b_proj
    return out


_NC_CACHE = {}


def kernel(x, W_qkv, b_qkv, W_proj, b_proj):
    if T not in _NC_CACHE:
        _NC_CACHE[T] = build_nc(T)
    nc = _NC_CACHE[T]
    in_maps = host_prep(x, W_qkv, b_qkv, W_proj, b_proj)
    res = run_bass_kernel_spmd(nc, in_maps, core_ids=list(range(N_CORES)))
    return host_gather(res.results, b_proj)


# revision 21
# speedup vs baseline: 40.3058x; 40.3058x over previous
"""Causal multi-head attention on 8 Trainium2 cores (raw Bass).

Problem: x[4,2048,1024] @ W_qkv -> 16-head causal attention -> @ W_proj.
Sharding: core c handles batch b=c//2 and head-half c%2 (8 heads each).
Host pre-transposes x (feature-major xT, bf16) and pre-slices/scales
weights; each core computes its heads' contribution to out^T; host sums
the two half contributions per batch and adds b_proj.

v2 pipeline (bf16 attention operands, fp32 PSUM, head-PAIR waves):
  The 8 heads form 4 pairs g; head h=2g lives on partitions 0:64 and
  h=2g+1 on 64:128 of qk_sb.  Per (g, q-chunk) the S^T matmuls of both
  heads issue back-to-back with 64-row operands at base partition 0/64,
  so the PE runs them CONCURRENTLY on disjoint row-groups (2x on S).
  exp on ACT covers both heads' PSUM banks in one instruction; DVE
  applies the causal triangle on diagonal 128-blocks post-exp (bf16 4x
  mode).  PV accumulates y_aug^T per head (ones-column gives softmax
  sums), reciprocal + K=1 replication matmuls + DVE multiply normalize.

  Cross-phase software pipelining: A1 (q,k projection) is emitted
  per-pair; pair g+1's A1 matmuls are interleaved into pair g's
  attention waves as PE filler (PSUM banks 6/7, shared with the
  replication matmuls via WAR semaphores) so the PE never stalls while
  ACT catches up on exp.  The C phase (out-proj) units for q-chunks
  0..2 interleave into the last pair's attention the same way.

build_nc(t, reps) can replicate the whole pipeline `reps` times inside
one NEFF (serialized at rep boundaries) for wall-clock timing dilation.
"""

import contextlib
import math

import numpy as np
import ml_dtypes

import concourse.bass as bass
import concourse.mybir as mybir
from concourse.bass_utils import run_bass_kernel_spmd

F32 = mybir.dt.float32
F32R = mybir.dt.float32r
BF16 = mybir.dt.bfloat16
ADD = mybir.AluOpType.add
MULT = mybir.AluOpType.mult
EXP = mybir.ActivationFunctionType.Exp
COPY = mybir.ActivationFunctionType.Copy

D_MODEL = 1024
D_K = 64
B, T = 4, 2048
NH = 8          # heads per core
NP = 4          # head pairs per core
KC = 8          # D_MODEL / 128
TQ = 512        # q-chunk width
N_CORES = 8


def build_nc(t=T, reps=1):
    tt_n = t // 128
    tc_n = t // TQ
    nc = bass.Bass(target_bir_lowering=False)

    xT_d = nc.dram_tensor("xT", [128, KC, t], BF16, kind="ExternalInput")
    wqk_d = nc.dram_tensor("wqk", [128, KC, NP, 2, 128], BF16,
                           kind="ExternalInput")
    wv_d = nc.dram_tensor("wv", [128, KC, 512], BF16, kind="ExternalInput")
    wproj_d = nc.dram_tensor("wproj", [128, 4, 1024], F32R,
                             kind="ExternalInput")
    bqk_d = nc.dram_tensor("bqk", [128, 8], F32, kind="ExternalInput")
    bv_d = nc.dram_tensor("bv", [128, 512], F32, kind="ExternalInput")
    tri_d = nc.dram_tensor("tri", [128, 128], BF16, kind="ExternalInput")
    ones_d = nc.dram_tensor("onesv", [128, 64], F32R, kind="ExternalInput")
    out_d = nc.dram_tensor("outT", [128, 8, t], F32, kind="ExternalOutput")

    # ---- schedule state ----
    prog = {"sync": [], "tensor": [], "vector": [], "scalar": []}
    cnt = {"pe": 0, "act": 0, "dve": 0}
    for _c in range(8):
        cnt[f"dma{_c}"] = 0
    last_wait = {e: {} for e in prog}
    bank_war = {}          # psum bank -> (sem, value): last consumer finished
    FUSE = {"tensor", "vector", "scalar"}

    def op(engine, fn, waits=(), incs=(), no_fuse=False):
        w = []
        for s, v in waits:
            if v <= 0 or last_wait[engine].get(s, -1) >= v:
                continue
            last_wait[engine][s] = v
            w.append((s, v))
        prog[engine].append((fn, w, list(incs),
                             engine in FUSE and not no_fuse))
        for s, a in incs:
            cnt[s] += a

    NDMA = 8
    dma_rr = [0]

    def dma(dst, src, waits=()):
        ch = dma_rr[0] % NDMA
        dma_rr[0] += 1
        sem = f"dma{ch}"
        w = [(sem, cnt[sem])] + list(waits)   # chain within channel
        op("sync", lambda e, d=dst, s=src: e.dma_start(d, s),
           w, [(sem, 16)])
        return (sem, cnt[sem])

    stack = contextlib.ExitStack()
    sb = lambda name, shape, dt: stack.enter_context(
        nc.sbuf_tensor(name, shape, dt))

    xT_sb = sb("xT_sb", [128, KC, t], BF16)
    wqg = sb("wqg", [128, NP, KC, 2, 128], BF16)
    wv_buf = sb("wv_buf", [128, 4, 512], BF16)
    qk_sb = sb("qk", [128, 8, t], BF16)          # f: 0..3 q-pair, 4..7 k-pair
    v_sb = sb("vsb", [128, tt_n, 8, 65], BF16)
    pt_sb = sb("pt", [128, 8, 512], BF16)        # slot = 2*(kt%4) + head
    rsb = sb("rsb", [65, 2, 512], F32R)
    yun = sb("yun", [64, 2, 512], F32R)
    ysb = sb("ysb", [128, 4, t], F32R)
    ysbt = sb("ysbt", [64, 2, t], F32R)
    osb = sb("osb", [128, 8, 512], F32)
    wproj_sb = sb("wproj_sb", [128, 4, 1024], F32R)
    bqk_sb = sb("bqk_sb", [128, 8], F32)
    bv_sb = sb("bv_sb", [128, 512], F32)
    tri_sb = sb("tri_sb", [128, 128], BF16)
    ones_sb = sb("ones_sb", [128, 64], F32R)
    psum = stack.enter_context(nc.psum_tensor("ps", [128, 8, 512], F32))

    with contextlib.ExitStack() as semstack:
        semstack.enter_context(nc.allow_low_precision(
            reason="bf16 matmul operands are intentional (checked rel err)"))
        sems = {}
        for _nm in ["pe", "act", "dve"] + [f"dma{_c}" for _c in range(8)]:
            sems[_nm] = semstack.enter_context(nc.semaphore(_nm + "_sem"))

        bqk_ret = dma(bqk_sb.ap(), bqk_d[:])
        bv_ret = dma(bv_sb.ap(), bv_d[:])
        tri_ret = dma(tri_sb.ap(), tri_d[:])
        ones_ret = dma(ones_sb.ap(), ones_d[:])

        # ones column of V_aug via DVE (x*0 + 1)
        op("vector",
           lambda e: e.tensor_scalar(
               v_sb.ap()[:, :, :, 64:65],
               bv_sb.ap()[:, 0:tt_n * 8].rearrange(
                   "p (a b c) -> p a b c", a=tt_n, b=8),
               0.0, 1.0, MULT, mybir.AluOpType.add),
           [bv_ret], [("dve", 1)])
        vones_ret = ("dve", cnt["dve"])

        ysbt_war = {}
        rep_gate = []

        for rep in range(reps):
            # ---- per-rep input DMAs ----
            xT_done = {}
            for kc in range(KC):
                xT_done[kc] = dma(xT_sb.ap()[:, kc], xT_d[:, kc],
                                  list(rep_gate))
            wqg_done = {}
            for g in range(NP):
                for kc in range(KC):
                    wqg_done[(g, kc)] = dma(
                        wqg.ap()[:, g, kc], wqk_d[:, kc, g], list(rep_gate))

            # ---------- A1 emission helper (one chain = 8 kc MMs + copy) ---
            qk_done = {}

            def a1_chain(g, tc, j, bank):
                """Emit full chain immediately (A-phase mode)."""
                for kc in range(KC):
                    a1_mm(g, tc, j, bank, kc)
                a1_copy(g, tc, j, bank)

            def a1_mm(g, tc, j, bank, kc):
                # weight (wqg) RAW waits must be pre-emitted wait_ge so
                # LDWEIGHTS cannot run before them -> no_fuse below.
                w = [xT_done[kc], wqg_done[(g, kc)]]
                if kc == 0 and bank in bank_war:
                    w.append(bank_war.pop(bank))
                op("tensor",
                   lambda e, b=bank, g_=g, k=kc, j_=j, tc_=tc:
                       e.matmul(
                           psum.ap()[:, b],
                           wqg.ap()[:, g_, k, j_],
                           xT_sb.ap()[:, k, tc_ * TQ:(tc_ + 1) * TQ],
                           start=(k == 0), stop=(k == KC - 1)),
                   w, [("pe", 1)] if kc == KC - 1 else [], no_fuse=True)

            def a1_copy(g, tc, j, bank):
                f = g + 4 * j
                mm_done = cnt["pe"]
                op("vector",
                   lambda e, b=bank, f_=f, tc_=tc:
                       e.tensor_scalar(
                           qk_sb.ap()[:, f_, tc_ * TQ:(tc_ + 1) * TQ],
                           psum.ap()[:, b],
                           bqk_sb.ap()[:, f_:f_ + 1], None, ADD),
                   [("pe", mm_done), bqk_ret], [("dve", 1)])
                bank_war[bank] = ("dve", cnt["dve"])
                qk_done[g] = cnt["dve"]

            # ---- A1 for pair 0 (dedicated phase, banks 0..3) ----
            for tc in range(tc_n):
                for j in range(2):
                    a1_chain(0, tc, j, 2 * (tc % 2) + j)

            # ---- A2: V projection, all pairs (baseline structure, bf16) ---
            tt_groups = [list(range(i, min(i + 4, tt_n)))
                         for i in range(0, tt_n, 4)]
            wv_seq = 0
            a2_kc_done = {}
            for tg, tts in enumerate(tt_groups):
                for kc in range(KC):
                    slot = wv_seq % 4
                    war = list(rep_gate)
                    if wv_seq - 4 >= 0:
                        war.append(("pe", a2_kc_done[wv_seq - 4]))
                    nd = dma(wv_buf.ap()[:, slot], wv_d[:, kc], war)
                    for j, tt in enumerate(tts):
                        bank = (tg % 2) * 4 + j
                        w = [nd, xT_done[kc]] if j == 0 else [xT_done[kc]]
                        if kc == 0 and bank in bank_war:
                            w.append(bank_war.pop(bank))
                        op("tensor",
                           lambda e, b=bank, s=slot, k=kc, tt_=tt:
                               e.matmul(
                                   psum.ap()[:, b],
                                   xT_sb.ap()[:, k, tt_ * 128:(tt_ + 1) * 128],
                                   wv_buf.ap()[:, s],
                                   start=(k == 0), stop=(k == KC - 1)),
                           w, [("pe", 1)] if j == len(tts) - 1 else [])
                    a2_kc_done[wv_seq] = cnt["pe"]
                    wv_seq += 1
                grp_done = cnt["pe"]
                for j, tt in enumerate(tts):
                    bk = (tg % 2) * 4 + j
                    op("vector",
                       lambda e, b=bk, tt_=tt:
                           e.tensor_tensor(
                               v_sb.ap()[:, tt_, :, 0:64],
                               psum.ap()[:, b], bv_sb.ap()[:], ADD),
                       [("pe", grp_done), bv_ret], [("dve", 1)])
                    bank_war[bk] = ("dve", cnt["dve"])
            a2_copies = cnt["dve"]
            a2_pe_done = cnt["pe"]

            wproj_dma = dma(wproj_sb.ap(), wproj_d[:],
                            [("pe", a2_pe_done)] + list(rep_gate))

            # ---------- filler queue (A1 of pair g+1, or C units) ----------
            # items are (fn, chain_end); a PSUM-bank accumulation chain must
            # never straddle another matmul on its bank, so finish_chain()
            # force-emits to the next chain boundary before rep matmuls.
            filler_items = []

            def load_a1_filler(g):
                # chains alternate banks 6/7; copy emitted with 8th MM
                for ci, (tc, j) in enumerate(
                        [(tc, j) for tc in range(tc_n) for j in range(2)]):
                    bank = 6 + (ci % 2)
                    for kc in range(KC):
                        def item(g=g, tc=tc, j=j, bank=bank, kc=kc):
                            a1_mm(g, tc, j, bank, kc)
                            if kc == KC - 1:
                                a1_copy(g, tc, j, bank)
                        filler_items.append((item, kc == KC - 1))

            mid_chain = [False]

            def emit_filler(n):
                for _ in range(n):
                    if not filler_items:
                        return
                    fn, end = filler_items.pop(0)
                    fn()
                    mid_chain[0] = not end

            def finish_chain():
                while mid_chain[0] and filler_items:
                    fn, end = filler_items.pop(0)
                    fn()
                    mid_chain[0] = not end

            # ---------- C-phase unit ----------
            c_copy = {}
            c_dma = {}
            c_seq = [0]
            norm_gate = {}        # tc -> (dve cnt, [shift rets]) needed by C

            def c_unit(ft, tc, bank):
                j = c_seq[0]
                dv, shifts = norm_gate[tc]
                w = [("dve", dv), wproj_dma] + shifts
                if bank in bank_war:
                    w.append(bank_war.pop(bank))
                if j >= 8:
                    w.append(("act", c_copy[j - 8]))
                for gg in range(4):
                    op("tensor",
                       lambda e, bk=bank, g_=gg, f=ft, tc_=tc: e.matmul(
                           psum.ap()[:, bk],
                           wproj_sb.ap()[:, g_, f * 128:(f + 1) * 128],
                           ysb.ap()[:, g_, tc_ * TQ:(tc_ + 1) * TQ],
                           start=(g_ == 0), stop=(g_ == 3)),
                       w if gg == 0 else [],
                       [("pe", 1)] if gg == 3 else [])
                mm_done = cnt["pe"]
                w = [("pe", mm_done)]
                if j >= 8:
                    w.append(c_dma[j - 8])
                op("scalar",
                   lambda e, bk=bank, ob=j % 8: e.activation(
                       osb.ap()[:, ob], psum.ap()[:, bk], COPY),
                   w, [("act", 1)])
                c_copy[j] = cnt["act"]
                bank_war[bank] = ("act", cnt["act"])
                c_dma[j] = dma(
                    out_d[:, ft, tc * TQ:(tc + 1) * TQ],
                    osb.ap()[:, j % 8],
                    [("act", c_copy[j])])
                c_seq[0] += 1

            def load_c_filler(tc):
                for fi, ft in enumerate(range(8)):
                    def item(ft=ft, tc=tc, fi=fi):
                        c_unit(ft, tc, 6 + (fi % 2))
                    filler_items.append((item, True))

            # ---------- B phase: head pairs ----------
            s_exp = {}            # kt -> act cnt of exp
            slot_war = {}         # pt slot -> pe cnt of last PV read
            tri_done = {}         # (0|1) -> dve cnt  (r01 / r23 masks)
            ybank_war = {}        # pv bank -> (sem, val)
            shifts_by_tc = {}     # tc -> [shift dma rets] across pairs
            first_pv = [True]

            for g in range(NP):
                while filler_items:
                    emit_filler(8)
                mid_chain[0] = False
                if g < NP - 1:
                    load_a1_filler(g + 1)
                for qc in range(tc_n):
                    nkt = 4 * qc + 4
                    s_exp.clear()
                    tri_done.clear()

                    def s_wave(kt, qc=qc, g=g):
                        bA = 2 * (kt % 2)
                        r = kt - 4 * qc
                        off = max(0, r * 128)
                        n = TQ - off
                        # qk RAW waits must be pre wait_ge (LDWEIGHTS runs
                        # ahead of any wait fused onto the MATMUL) -> no_fuse
                        w = [("dve", qk_done[g])]
                        for b in (bA, bA + 1):
                            if b in bank_war:
                                w.append(bank_war.pop(b))
                        if kt - 2 in s_exp:
                            w.append(("act", s_exp[kt - 2]))
                        for hh in (0, 1):
                            qrow = hh * 64
                            op("tensor",
                               lambda e, b=bA + hh, qrow=qrow, kt=kt,
                                      off=off, n=n, g=g:
                                   e.matmul(
                                       psum.ap()[:, b, off:off + n],
                                       qk_sb.ap()[qrow:qrow + 64, 4 + g,
                                                  kt * 128:(kt + 1) * 128],
                                       qk_sb.ap()[qrow:qrow + 64, g,
                                                  qc * TQ + off:
                                                  qc * TQ + off + n],
                                       start=True, stop=True),
                               w if hh == 0 else [],
                               [("pe", 1)] if hh == 1 else [],
                               no_fuse=True)
                        s_done = cnt["pe"]
                        # exp over both heads' banks -> pt slots
                        sl = 2 * (kt % 4)
                        w = [("pe", s_done)]
                        if sl in slot_war:
                            w.append(("pe", slot_war[sl]))
                        op("scalar",
                           lambda e, bA=bA, sl=sl, off=off, n=n:
                               e.activation(
                                   pt_sb.ap()[:, sl:sl + 2, off:off + n],
                                   psum.ap()[:, bA:bA + 2, off:off + n],
                                   EXP),
                           w, [("act", 1)])
                        s_exp[kt] = cnt["act"]
                        bank_war[bA] = ("act", cnt["act"])
                        bank_war[bA + 1] = ("act", cnt["act"])
                        # triangle masks after exps of diag rows r=0,1 / 2,3
                        if r == 1 or r == 3:
                            half = r // 2
                            for rr in (r - 1, r):
                                for hh in (0, 1):
                                    slr = 2 * ((4 * qc + rr) % 4) + hh
                                    op("vector",
                                       lambda e, s=slr, rr=rr:
                                           e.tensor_tensor(
                                               pt_sb.ap()[:, s, rr * 128:
                                                          rr * 128 + 128],
                                               pt_sb.ap()[:, s, rr * 128:
                                                          rr * 128 + 128],
                                               tri_sb.ap()[:], MULT),
                                       [("act", s_exp[4 * qc + rr]), tri_ret],
                                       [("dve", 1)])
                            tri_done[half] = cnt["dve"]

                    def pv_pair(kt, qc=qc, g=g):
                        r = kt - 4 * qc
                        off = max(0, r * 128)
                        n = TQ - off
                        w = [("act", s_exp[kt])]
                        if r >= 0:
                            w.append(("dve", tri_done[max(0, r) // 2]))
                        if first_pv[0]:
                            w += [vones_ret, ("dve", a2_copies)]
                            first_pv[0] = False
                        if kt == 0:
                            for b in (4, 5):
                                if b in ybank_war:
                                    w.append(ybank_war[b])
                        for hh in (0, 1):
                            sl = 2 * (kt % 4) + hh
                            op("tensor",
                               lambda e, yb=4 + hh, h=2 * g + hh, sl=sl,
                                      kt=kt, off=off, n=n, nkt=nkt:
                                   e.matmul(
                                       psum.ap()[0:65, yb, off:off + n],
                                       v_sb.ap()[:, kt, h, :],
                                       pt_sb.ap()[:, sl, off:off + n],
                                       start=(kt == 0), stop=(kt == nkt - 1)),
                               w if hh == 0 else [],
                               [("pe", 1)] if hh == 1 else [])
                        slot_war[2 * (kt % 4)] = cnt["pe"]
                        slot_war[2 * (kt % 4) + 1] = cnt["pe"]

                    for kt in range(nkt):
                        emit_filler(2)
                        s_wave(kt)
                        if kt >= 2:
                            pv_pair(kt - 2)
                    emit_filler(2)
                    pv_pair(nkt - 2)
                    emit_filler(2)
                    pv_pair(nkt - 1)
                    pv_all = cnt["pe"]

                    # softmax sums -> reciprocal (both heads, one DVE op)
                    op("vector",
                       lambda e: e.reciprocal(
                           rsb.ap()[64:65, :, :], psum.ap()[64:65, 4:6, :]),
                       [("pe", pv_all)], [("dve", 1)])
                    recip_done = cnt["dve"]
                    # replication matmuls (banks 6,7 shared with filler)
                    finish_chain()
                    rep_pe = {}
                    for hh in (0, 1):
                        w = [("dve", recip_done), ones_ret]
                        if 6 + hh in bank_war:
                            w.append(bank_war.pop(6 + hh))
                        op("tensor",
                           lambda e, rb=6 + hh, hh=hh: e.matmul(
                               psum.ap()[0:64, rb],
                               ones_sb.ap()[64:65, :],
                               rsb.ap()[64:65, hh],
                               start=True, stop=True),
                           w, [("pe", 1)])
                        rep_pe[hh] = cnt["pe"]
                    # yun copies + normalize + per-qc ysbt shift
                    shift_w = []
                    for hh in (0, 1):
                        op("vector",
                           lambda e, yb=4 + hh, hh=hh: e.tensor_copy(
                               yun.ap()[0:64, hh].bitcast(F32),
                               psum.ap()[0:64, yb]),
                           [("pe", pv_all)], [("dve", 1)])
                        yun_cnt = cnt["dve"]
                        ybank_war[4 + hh] = ("dve", yun_cnt)
                        if hh == 0:
                            out_ap = ysb.ap()[0:64, g, qc * TQ:(qc + 1) * TQ]
                        else:
                            out_ap = ysbt.ap()[0:64, g % 2,
                                               qc * TQ:(qc + 1) * TQ]
                        w = [("pe", rep_pe[hh]), ("dve", yun_cnt)]
                        if hh == 1 and (g % 2, qc) in ysbt_war:
                            w.append(ysbt_war[(g % 2, qc)])
                        op("vector",
                           lambda e, o=out_ap, rb=6 + hh, hh=hh:
                               e.tensor_tensor(
                                   o, yun.ap()[0:64, hh].bitcast(F32),
                                   psum.ap()[0:64, rb], MULT),
                           w, [("dve", 1)])
                        bank_war[6 + hh] = ("dve", cnt["dve"])
                        if hh == 1:
                            nd = dma(ysb.ap()[64:128, g,
                                              qc * TQ:(qc + 1) * TQ],
                                     ysbt.ap()[0:64, g % 2,
                                               qc * TQ:(qc + 1) * TQ],
                                     [("dve", cnt["dve"])])
                            ysbt_war[(g % 2, qc)] = nd
                            shifts_by_tc.setdefault(qc, []).append(nd)
                    if g == NP - 1:
                        norm_gate[qc] = (cnt["dve"],
                                         list(shifts_by_tc[qc]))
                        if qc < tc_n - 1:
                            load_c_filler(qc)
                if g == NP - 1:
                    # drain any unemitted C filler, then tail tc
                    while filler_items:
                        emit_filler(8)

            # ---- C tail: last q-chunk on banks 0..3 ----
            for ft in range(8):
                c_unit(ft, tc_n - 1, ft % 4)

            rep_gate = [("act", c_copy[c_seq[0] - 1]), c_dma[c_seq[0] - 1]]
            # seed psum WARs for next rep (S banks last read by exp; PV by
            # yun copies; 6/7 by norms/C units)
            for bk in range(8):
                bank_war.setdefault(bk, rep_gate[0])

        # ---- emit ----
        with nc.Block() as block:
            def emitter(name):
                def run(eng):
                    for fn, waits, incs, fuse in prog[name]:
                        pre = waits[1:] if (fuse and waits) else waits
                        for s, v in pre:
                            eng.wait_ge(sems[s], v)
                        ins = fn(eng)
                        if fuse and waits:
                            s, v = waits[0]
                            ins.wait_op(sems[s], v, "sem-ge")
                        for s, a in incs:
                            ins.then_inc(sems[s], a)
                return run
            block.sync(emitter("sync"))
            block.tensor(emitter("tensor"))
            block.vector(emitter("vector"))
            block.scalar(emitter("scalar"))

    stack.close()
    return nc


# ---------------------------------------------------------------------------

def host_prep(x, W_qkv, b_qkv, W_proj, b_proj, t=T):
    scale = 1.0 / math.sqrt(D_K)
    x = np.asarray(x, np.float32)
    W_qkv = np.asarray(W_qkv, np.float32)
    b_qkv = np.asarray(b_qkv, np.float32)
    W_proj = np.asarray(W_proj, np.float32)
    bf = ml_dtypes.bfloat16

    tri = (np.arange(128)[None, :] >= np.arange(128)[:, None]) \
        .astype(bf)
    onesv = np.ones((128, 64), np.float32)

    in_maps = []
    for c in range(N_CORES):
        b = c // 2
        f0 = (c % 2) * 512
        xT = np.ascontiguousarray(
            x[b, :t].T.reshape(KC, 128, t).transpose(1, 0, 2)).astype(bf)
        wq = W_qkv[:, f0:f0 + 512] * scale
        wk = W_qkv[:, D_MODEL + f0:D_MODEL + f0 + 512]
        # [1024, NP, 2, 128]: per pair g the q slice then k slice
        wqk = np.stack([wq.reshape(D_MODEL, 4, 128),
                        wk.reshape(D_MODEL, 4, 128)], axis=2)
        wqk = np.ascontiguousarray(
            wqk.reshape(KC, 128, NP, 2, 128).transpose(1, 0, 2, 3, 4)
        ).astype(bf)
        wv = W_qkv[:, 2 * D_MODEL + f0:2 * D_MODEL + f0 + 512]
        wv = np.ascontiguousarray(
            wv.reshape(KC, 128, 512).transpose(1, 0, 2)).astype(bf)
        bq = b_qkv[f0:f0 + 512] * scale
        bk = b_qkv[D_MODEL + f0:D_MODEL + f0 + 512]
        bqk = np.ascontiguousarray(
            np.concatenate([bq, bk]).reshape(8, 128).T)
        bv = b_qkv[2 * D_MODEL + f0:2 * D_MODEL + f0 + 512]
        bv_rep = np.broadcast_to(bv, (128, 512)).copy()
        wp = W_proj[f0:f0 + 512]
        wp = np.ascontiguousarray(
            wp.reshape(4, 128, 1024).transpose(1, 0, 2))
        in_maps.append({
            "xT": xT, "wqk": wqk, "wv": wv, "wproj": wp,
            "bqk": bqk, "bv": bv_rep, "tri": tri, "onesv": onesv,
        })
    return in_maps


def host_gather(results, b_proj, t=T):
    b_proj = np.asarray(b_proj, np.float32)
    out = np.empty((B, t, D_MODEL), np.float32)
    for b in range(B):
        acc = None
        for half in range(2):
            r = results[2 * b + half]["outT"]
            oT = r.transpose(1, 0, 2).reshape(D_MODEL, t)
            acc = oT if acc is None else acc + oT
        out[b] = acc.T + b_proj
    return out


_NC_CACHE = {}


def kernel(x, W_qkv, b_qkv, W_proj, b_proj):
    if T not in _NC_CACHE:
        _NC_CACHE[T] = build_nc(T)
    nc = _NC_CACHE[T]
    in_maps = host_prep(x, W_qkv, b_qkv, W_proj, b_proj)
    res = run_bass_kernel_spmd(nc, in_maps, core_ids=list(range(N_CORES)))
    return host_gather(res.results, b_proj)
